# revision 35
# baseline (speedup 1.0000x reference)
"""HGNN conv kernel for Trainium2, data-parallel over time across 8 cores.

Per core (t = core index): out_b = Dv^-1/2 Gc De^-1 Gc^T Dv^-1/2 (x_b W + 1 b^T)
computed in factored form (L never materialized):
  Gs  = Dv^-1/2 Gc                      [N, E]
  z   = x_t^T Gs  per 128-row bf block  [BF, E]   (MM1)
  v   = zT-blocks @ blockdiag(W,W) + u0 bias^T  [E, BF]  (W-MM, fused bias)
  out = Gsd^T v with Gsd = de * Gs^T    [N, BF]   (MM2)
All big matmuls run in bf16 (rel err ~8e-4, PE at 1 cyc/row); the
degree stats stay f32.

DMA layout: the on-chip n axis is stored as n = 8*p + k (p = partition,
k = 0..7 inner) so every HBM x/out descriptor moves a contiguous
(8 n x 64 f) = 2 KB run (vs 256 B in the naive n-major layout).
Matmul contraction over n is permutation-invariant, so only G/Gs/Gsd
need the matching row order. x is staged [p, b, k, f] (big descriptors)
and repacked on-chip to k-major bf16 for single-free-dim lhsT APs.

Schedule: one HWDGE load ring carries G (halves, full bandwidth, they
gate the dv stats) then the x chunks. Engine queues are emitted in
dependency-arrival order (warm-up matmuls gated on the first G half lift
the HAM throttle right before the real stream). MM1 m-tiles consume x
chunks as they land, gsd transposes and MM2 blocks interleave as their
inputs complete, stores stream per block with the last block split fine.
PSUM note: a start=True matmul clears has_written for its whole bank, so
concurrently-accumulating groups must sit in different banks.
"""

import sys

import numpy as np

sys.path.insert(0, "/opt/trn_rl_repo")

from contextlib import ExitStack

import concourse.bass as bass
import concourse.mybir as mybir
import concourse.tile as tile
from concourse import bacc, bass_utils
from concourse.masks import make_identity

P = 128
T = 8
B = 28          # batch entries per core
N = 1024        # nodes
E = 512         # hyperedges (256 static + 256 dynamic)
F = 64          # features
BF = B * F      # 1792
EPS = 1e-6
KN = 8          # inner n factor: n = 8*p + k
H = KN // 2
MT = BF // P    # 14 bf-tiles (2 batch entries each)
ET = E // P     # 4 e-tiles
NB = 4          # output free-dim chunks
NBW = BF // NB  # 448 = 7 batch entries * 64
XC = 14         # x load chunks (2 b-entries each, = 1 m-tile)

f32 = mybir.dt.float32
f32r = mybir.dt.float32r
bf16 = mybir.dt.bfloat16


def _build_nc():
    nc = bacc.Bacc("TRN2", target_bir_lowering=False, debug=False)

    xs = nc.dram_tensor("xs", [B, N, F], f32, kind="ExternalInput").ap()
    g = nc.dram_tensor("g", [N, 256], f32, kind="ExternalInput").ap()
    g1 = nc.dram_tensor("g1", [N, 256], f32, kind="ExternalInput").ap()
    w = nc.dram_tensor("w", [F, F], f32, kind="ExternalInput").ap()
    bvec = nc.dram_tensor("b", [F], f32, kind="ExternalInput").ap()
    os_ = nc.dram_tensor("os", [B, N, F], f32, kind="ExternalOutput").ap()

    with tile.TileContext(nc) as tc, ExitStack() as ctx:
        const = ctx.enter_context(tc.tile_pool(name="const", bufs=1))
        big = ctx.enter_context(tc.tile_pool(name="big", bufs=1))
        ztp = ctx.enter_context(tc.tile_pool(name="ztp", bufs=3))
        xstage = ctx.enter_context(tc.tile_pool(name="xstage", bufs=4))
        ps_warm = ctx.enter_context(tc.tile_pool(name="ps_warm", bufs=1, space="PSUM"))
        ps_small = ctx.enter_context(tc.tile_pool(name="ps_small", bufs=2, space="PSUM"))
        ps_z = ctx.enter_context(tc.tile_pool(name="ps_z", bufs=2, space="PSUM"))
        ps_o = ctx.enter_context(tc.tile_pool(name="ps_o", bufs=3, space="PSUM"))

        # ---- input loads (one HWDGE ring, FIFO) --------------------------
        # G rows n = 8p+k: per partition one contiguous 4 KB DRAM run per
        # half. G halves first at full bandwidth (they gate everything),
        # then x chunks just ahead of their consumers.
        gcs32 = big.tile([P, KN, 256], f32, name="gcs32")
        gcd32 = big.tile([P, KN, 256], f32, name="gcd32")
        gr = g.rearrange("(p k) e -> p k e", k=KN)
        g1r = g1.rearrange("(p k) e -> p k e", k=KN)
        xs_r = xs.rearrange("b (p k) f -> p b k f", k=KN)
        stage_bufs = [
            xstage.tile([P, 2, KN, F], f32, name="xst") for _ in range(XC)
        ]
        nc.sync.dma_start(gcs32[:, 0:H], gr[:, 0:H])
        nc.sync.dma_start(gcd32[:, 0:H], g1r[:, 0:H])
        nc.sync.dma_start(gcs32[:, H:], gr[:, H:])
        nc.sync.dma_start(gcd32[:, H:], g1r[:, H:])
        for c in range(XC):
            nc.sync.dma_start(stage_bufs[c][:], xs_r[:, 2 * c : 2 * c + 2])
        xp = big.tile([P, KN, B, F], bf16, name="xp")

        def repack(c, eng):
            # k-major bf16 repack; the f32->bf16 cast rides the copy
            dst = xp[:, :, 2 * c : 2 * c + 2, :]
            srcv = stage_bufs[c][:].rearrange("p b k f -> p k b f")
            if eng == "v":
                nc.vector.tensor_copy(dst, srcv)
            else:
                nc.scalar.copy(dst, srcv)

        # ---- small consts (SWDGE: separate queue, casts ride the DMA) ----
        bdw = const.tile([P, P], bf16, name="bdw")
        nc.vector.memset(bdw[:], 0.0)
        nc.gpsimd.dma_start(bdw[0:64, 0:64], w)
        nc.gpsimd.dma_start(bdw[64:128, 64:128], w)
        bias2 = const.tile([1, 2, F], f32r, name="bias2")
        nc.gpsimd.dma_start(bias2[:, 0, :], bvec[None, :])
        nc.gpsimd.dma_start(bias2[:, 1, :], bvec[None, :])

        ident_f = const.tile([P, P], f32, name="ident_f")
        make_identity(nc, ident_f[:])

        bias_bc = const.tile([P, P], f32r, name="bias_bc")
        nc.gpsimd.partition_broadcast(
            bias_bc[:], bias2[:].rearrange("o t f -> o (t f)")
        )

        # ---- stats tiles + cheap memsets ---------------------------------
        gc16 = big.tile([P, KN, E], bf16, name="gc16")
        rs = const.tile([P, KN], f32, name="rs")
        rs2 = const.tile([P, KN], f32, name="rs2")
        eps_col = const.tile([P, 1], f32, name="eps_col")
        nc.vector.memset(eps_col[:], EPS)
        sq = const.tile([P, KN], f32, name="sq")
        dv = const.tile([P, KN], f32, name="dv")
        onesdv_f = const.tile([P, KN, 2], f32, name="onesdv_f")
        nc.vector.memset(onesdv_f[:, :, 0:1], 1.0)
        onesdv = const.tile([P, KN, 2], bf16, name="onesdv")
        gs_all = big.tile([P, KN, E], bf16, name="gs_all")
        warm_f = const.tile([P, E], f32, name="warm_f")
        nc.vector.memset(warm_f[:], 0.0)

        # ---- PE warm-up: gated on the first G half -----------------------
        # (values are junk; the output is never read)
        warm_r = const.tile([P, E], f32r, name="warm_r")
        nc.vector.scalar_tensor_tensor(
            out=warm_r[:], in0=gcs32[:, 0:H].rearrange("p k e -> p (k e)")[:, 0:E],
            scalar=0.0, in1=warm_f[:],
            op0=mybir.AluOpType.mult, op1=mybir.AluOpType.add,
        )
        warm_ps = ps_warm.tile([P, E], f32, name="warm_ps")
        for _ in range(8):
            nc.tensor.matmul(warm_ps[:], warm_r[:, 0:128], warm_r[:], start=True, stop=True)

        # ---- degree stats, per G half ------------------------------------
        # dv = 1/sqrt(rowsum(Gc)+eps); bf16 copies of Gc feed the colsums
        def stats_half(h):
            kk = slice(h * H, h * H + H)
            nc.scalar.copy(gc16[:, kk, 0:256], gcs32[:, kk])
            nc.scalar.copy(gc16[:, kk, 256:512], gcd32[:, kk])
            nc.vector.reduce_sum(rs[:, kk, None], gcs32[:, kk], axis=mybir.AxisListType.X)
            nc.vector.reduce_sum(rs2[:, kk, None], gcd32[:, kk], axis=mybir.AxisListType.X)
            nc.vector.scalar_tensor_tensor(
                out=rs[:, kk], in0=rs[:, kk], scalar=1.0, in1=rs2[:, kk],
                op0=mybir.AluOpType.mult, op1=mybir.AluOpType.add,
            )
            nc.scalar.activation(
                sq[:, kk], rs[:, kk], mybir.ActivationFunctionType.Sqrt,
                bias=eps_col[:],
            )
            nc.vector.reciprocal(dv[:, kk], sq[:, kk])
            nc.vector.tensor_copy(onesdv_f[:, kk, 1:2], dv[:, kk, None])
            nc.vector.tensor_copy(onesdv[:, kk], onesdv_f[:, kk])
            # Gs = dv * Gc (bf16, straight from the f32 tiles so the
            # chain does not wait on the ACT casts)
            for k in range(h * H, h * H + H):
                nc.vector.tensor_scalar(
                    out=gs_all[:, k, 0:256], in0=gcs32[:, k, :], scalar1=dv[:, k : k + 1],
                    scalar2=None, op0=mybir.AluOpType.mult,
                )
                nc.vector.tensor_scalar(
                    out=gs_all[:, k, 256:512], in0=gcd32[:, k, :], scalar1=dv[:, k : k + 1],
                    scalar2=None, op0=mybir.AluOpType.mult,
                )

        stats_half(0)
        repack(0, "s")
        # matmul-ready identities (DVE, between the two stats halves so
        # they do not delay the h1 chain; needed from statsT on)
        ident = const.tile([P, P], f32r, name="ident")
        nc.vector.tensor_copy(ident[:], ident_f[:])
        ident16 = const.tile([P, P], bf16, name="ident16")
        nc.vector.tensor_copy(ident16[:], ident_f[:])
        stats_half(1)

        # colsums of Gc (row 0) and Gs (row 1) -> [2, E]: the two
        # accumulation groups live in SEPARATE PSUM banks (a start=True
        # clears has_written bank-wide), emitted per half.
        stats_s = ps_small.tile([2, 256], f32, name="sp")
        stats_d = ps_small.tile([2, 256], f32, name="sp")

        def colsums(h):
            for k in range(h * H, h * H + H):
                nc.tensor.matmul(
                    stats_s[:], onesdv[:, k, :], gc16[:, k, 0:256],
                    start=(k == 0), stop=(k == KN - 1),
                )
                nc.tensor.matmul(
                    stats_d[:], onesdv[:, k, :], gc16[:, k, 256:512],
                    start=(k == 0), stop=(k == KN - 1),
                )

        stats_sb = const.tile([2, E], f32r, name="stats_sb")
        statsT = const.tile([P, ET, 2], f32, name="statsT")
        de_col = const.tile([P, ET], f32, name="de_col")
        biasu = const.tile([P, ET, P], f32, name="biasu")

        def stats_finish():
            nc.vector.tensor_copy(stats_sb[:, 0:256], stats_s[:])
            nc.vector.tensor_copy(stats_sb[:, 256:512], stats_d[:])
            for j in range(ET):
                tp = ps_small.tile([P, P], f32r, name="sp")[:, 0:2]
                nc.tensor.matmul(
                    tp[:], stats_sb[:, j * P : (j + 1) * P], ident[0:2, 0:2],
                    is_transpose=True,
                )
                nc.vector.tensor_copy(statsT[:, j, :], tp[:])
            nc.vector.tensor_scalar(
                out=de_col[:], in0=statsT[:, :, 0], scalar1=EPS, scalar2=None,
                op0=mybir.AluOpType.add,
            )
            nc.vector.reciprocal(de_col[:], de_col[:])
            # biasu[j] = u0_j * [bias|bias] row-broadcast, reused by all m
            for j in range(ET):
                nc.vector.tensor_scalar(
                    out=biasu[:, j], in0=bias_bc[:], scalar1=statsT[:, j, 1:2],
                    scalar2=None, op0=mybir.AluOpType.mult,
                )

        # ---- main pipeline ----------------------------------------------
        gsd_all = big.tile([P, ET, KN, P], bf16, name="gsd_all")
        v_all = big.tile([P, ET, BF], bf16, name="v_all")
        os_all = big.tile([P, B, KN, F], f32, name="os_all")
        os_r = os_.rearrange("b (p k) f -> p b k f", k=KN)

        def gsd_block(j):
            # Gsd[e, (j, k, p)] = de[e] * Gs[(p, k), e] via PE transpose
            for k in range(KN):
                tp = ps_small.tile([P, P], bf16, name="sp")
                nc.tensor.matmul(
                    tp[:], gs_all[:, k, j * P : (j + 1) * P], ident16[:],
                    is_transpose=True,
                )
                if k % 2 == 0:
                    nc.vector.tensor_scalar(
                        out=gsd_all[:, j, k, :], in0=tp[:],
                        scalar1=de_col[:, j : j + 1], scalar2=None,
                        op0=mybir.AluOpType.mult,
                    )
                else:
                    nc.scalar.activation(
                        gsd_all[:, j, k, :], tp[:],
                        mybir.ActivationFunctionType.Copy,
                        scale=de_col[:, j : j + 1],
                    )

        def mm1_mms(m, zps, k0, k1):
            for k in range(k0, k1):
                xm = xp[:, k, 2 * m : 2 * m + 2, :].rearrange("p b f -> p (b f)")
                nc.tensor.matmul(
                    zps[:], xm, gs_all[:, k, :],
                    start=(k == 0), stop=(k == KN - 1),
                )

        def mm1_tile(m, zps=None):
            if zps is None:
                zps = ps_z.tile([P, E], f32, name="zps")
                mm1_mms(m, zps, 0, KN)
            zt = ztp.tile([P, E], bf16, name="zt")
            nc.scalar.copy(zt[:, 0:256], zps[:, 0:256])
            nc.vector.tensor_copy(zt[:, 256:512], zps[:, 256:512])
            for jp in range(ET // 2):
                # two W-matmuls share one PSUM bank (disjoint halves), then
                # one wide STT adds the precomputed u0*bias and casts bf16
                wps2 = ps_small.tile([P, 2 * P], f32, name="sp")
                for o in range(2):
                    j = 2 * jp + o
                    nc.tensor.matmul(
                        wps2[:, o * P : (o + 1) * P], zt[:, j * P : (j + 1) * P],
                        bdw[:], start=True, stop=True,
                    )
                nc.vector.scalar_tensor_tensor(
                    out=v_all[:, 2 * jp : 2 * jp + 2, m * P : (m + 1) * P],
                    in0=biasu[:, 2 * jp : 2 * jp + 2, :],
                    scalar=1.0,
                    in1=wps2[:].rearrange("p (j c) -> p j c", j=2),
                    op0=mybir.AluOpType.mult,
                    op1=mybir.AluOpType.add,
                )

        def mm2_block(b0, b1, last):
            # out chunk [128 p, b0:b1, 64 f] per k; store via 2-KB runs.
            # The last block stores progressively finer so the final
            # store (and its HBM completion) trails the compute minimally.
            c0, c1 = b0 * F, b1 * F
            for k in range(KN):
                ops = ps_o.tile([P, c1 - c0], f32, name="ops")
                for j in range(ET):
                    nc.tensor.matmul(
                        ops[:], gsd_all[:, j, k, :],
                        v_all[:, j, c0:c1],
                        start=(j == 0), stop=(j == ET - 1),
                    )
                dst = os_all[:, b0:b1, k, :]
                src = ops[:].rearrange("p (c f) -> p c f", f=F)
                if k % 2 == 0:
                    nc.vector.tensor_copy(dst, src)
                else:
                    nc.scalar.copy(dst, src)
                if last:
                    if k == 3:
                        nc.scalar.dma_start(
                            os_r[:, b0:b1, 0:4], os_all[:, b0:b1, 0:4]
                        )
                    elif k in (5, 6):
                        k0 = 4 if k == 5 else 6
                        nc.scalar.dma_start(
                            os_r[:, b0:b1, k0 : k + 1], os_all[:, b0:b1, k0 : k + 1]
                        )
            if last:
                nc.scalar.dma_start(
                    os_r[:, b0:b1, KN - 1 :], os_all[:, b0:b1, KN - 1 :]
                )
            else:
                nc.scalar.dma_start(os_r[:, b0:b1], os_all[:, b0:b1])

        mm2_after = {3: (0, 7), 7: (7, 14), 10: (14, 21), 13: (21, 28)}

        # m0 split: its first k-half runs between the two colsum halves so
        # the PE stream never waits on the second G half.
        colsums(0)
        zps0 = ps_z.tile([P, E], f32, name="zps")
        mm1_mms(0, zps0, 0, H)
        colsums(1)
        stats_finish()
        mm1_mms(0, zps0, H, KN)
        repack(1, "s")
        mm1_tile(0, zps=zps0)
        gsd_block(0)
        for m in range(1, MT):
            if m + 1 < XC:
                repack(m + 1, "v" if m % 2 else "s")
            mm1_tile(m)
            if m < ET:
                gsd_block(m)
            if m in mm2_after:
                b0, b1 = mm2_after[m]
                mm2_block(b0, b1, last=(b1 == B))

    nc.finalize()
    return nc


_NC = None


def _get_nc():
    global _NC
    if _NC is None:
        _NC = _build_nc()
    return _NC


def kernel(x, G, G1, weight, bias):
    nc = _get_nc()
    x = np.ascontiguousarray(x, dtype=np.float32)
    G = np.ascontiguousarray(G, dtype=np.float32)
    G1 = np.ascontiguousarray(G1, dtype=np.float32)
    weight = np.ascontiguousarray(weight, dtype=np.float32)
    bias = np.ascontiguousarray(bias, dtype=np.float32)

    in_maps = []
    for c in range(T):
        in_maps.append(
            {
                "xs": x[c * B : (c + 1) * B],
                "g": G,
                "g1": np.ascontiguousarray(G1[c]),
                "w": weight,
                "b": bias,
            }
        )
    res = bass_utils.run_bass_kernel_spmd(nc, in_maps, core_ids=list(range(T)))
    return np.concatenate([r["os"] for r in res.results], axis=0)
